# revision 30
# baseline (speedup 1.0000x reference)
"""Trainium2 Bass kernel for nn_CrossAttention2d (B=32, C=256, INNER=128, H=W=32).

Sharding: pure data parallel — batch 32 split as 4 items per core across 8
NeuronCores; all weights replicated. No collectives.

Per item (N = H*W = 1024 tokens, C = 256 channels, D = 128 inner), stream s
(s=0 -> fs output, s=1 -> fi output):
  q = wq[1-s] @ f[1-s], k = wk[s] @ f[s]   (fp8 DoubleRow, x32 prescale)
  vT[m, c] = (wv[s] @ f[s]).T              (fp8 DoubleRow, f-slices stationary)
  S^T[m, n] = sum_d k[d, m] q[d, n]        (bf16 PE, m-tiles of 128)
  E = exp(S^T / (1024 sqrt(D)))            (ACT, [128,1024] psum slab -> fp8)
  O_un[c, n] = sum_m vT[m, c] E[m, n]      (fp8 DR, two sequential C-half
                                            passes through the shared ring)
  den[n] via ones.T @ E (fp8 DR, rows all equal den[n])
  attn8 = O_un * (1/den)                   (DVE, = 32x true attn, fp8)
  fuse: g = relu((W1*1024 @ f  +  W2*32 @ attn8) / 1024 + b)
  h = g + f[s] (bf16 residual, split DVE/GpSimd); LayerNorm over (C,N).
  LN stats: DVE bn_stats records -> per-partition [sum m, sum M2, sum m^2]
  -> PE ones-colsum -> GpSimd scalar chain (Newton rsqrt) -> GpSimd apply
  out = h * A + B (bf16; host widens to f32).

Software pipeline, one iteration x per stream, pitch ~11.5us. Per-engine
queues in steady state (x = current stream):
  ACT : exp(x) x8 with relu(x-1) x2 and k-evac(x+1) slotted in the gaps
  DVE : recip(x-1), norm(x-1) x2, vt-evac(x) x2, q-evac(x+1),
        resid(x-1) t0, bn_stats(x-2)
  GpS : resid(x-1) t1, LN chain (odd x), apply(x-3) + out DMA doorbells
  PE  : S(x) x8, PV-t1(x-1), vt(x), fuse(x-1), q/k(x+1), PV-t0(x), den(x)

PSUM (8 banks): ring A tag 'slab' 2x[128,1024] = S slabs (+ tiny chain
matmul); ring B tag 'pvb' 2x[128,1024] rotating per iteration:
  psf_t0(x-1), psf_t1(x-1), q(x+1), k(x+1), den(x), pv_t0(x), pv_t1(x),
  vt0(x+1), vt1(x+1)
The bufs=2 WAR chain of ring B self-schedules the pipeline: e.g. pv_t0(x)
waits k(x+1) evac, psf_t0(x) waits vt0(x+1) evac, etc.

Matmul convention: out[M, N] = lhsT.T @ rhs, lhsT = [K<=128, M<=128] (K on
partitions), rhs = [K, N<=512], out in PSUM f32 (bank-contained writes).
DoubleRow: lhsT [Ki, 2, M], rhs [Ki, 2, N] fp8 -> contracts 2*Ki.
"""

import numpy as np
import ml_dtypes

import concourse.bacc as bacc
import concourse.bass as bass
import concourse.tile as tile
from concourse import mybir
from concourse.bass_utils import run_bass_kernel_spmd

F32 = mybir.dt.float32
BF16 = mybir.dt.bfloat16
FP8 = mybir.dt.float8e4
DR = mybir.MatmulPerfMode.DoubleRow
AF = mybir.ActivationFunctionType
OP = mybir.AluOpType
AX = mybir.AxisListType

B, C, D, N = 32, 256, 128, 1024
NCORES = 8
IPC = B // NCORES  # items per core = 4
NITER = 2 * IPC    # stream iterations per core = 8
WSCALE = 32.0  # fp8 weight prescale (w*32 keeps N(0,0.02) in e4m3 range)
EXP_SCALE = (1.0 / float(np.sqrt(D))) / (WSCALE * WSCALE)
EPS = 1e-5
NTOT = float(C * N)  # layernorm element count per item/stream

# test.py can set {"trace": True}; harness path leaves this empty.
RUN_KWARGS = {}
LAST_RESULT = None


def _build():
    nc = bacc.Bacc("TRN2", target_bir_lowering=False, debug=False,
                   num_devices=NCORES)

    # ---- DRAM I/O (per-core shapes) ----
    fb_d = [nc.dram_tensor(n_, [IPC, 128, 2, N], BF16, kind="ExternalInput")
            for n_ in ("fsb", "fib")]
    f8_d = [nc.dram_tensor(n_, [IPC, 128, 2, N], FP8, kind="ExternalInput")
            for n_ in ("fs8", "fi8")]
    wq_d = [nc.dram_tensor(n_, [128, 2, 128], FP8, kind="ExternalInput")
            for n_ in ("wq0", "wq1")]
    wk_d = [nc.dram_tensor(n_, [128, 2, 128], FP8, kind="ExternalInput")
            for n_ in ("wk0", "wk1")]
    wv_d = [nc.dram_tensor(n_, [128, 2, 256], FP8, kind="ExternalInput")
            for n_ in ("wv0", "wv1")]
    wf8_d = nc.dram_tensor("wfuse8", [128, 4, 256], FP8, kind="ExternalInput")
    fb_bias_d = nc.dram_tensor("fuseb", [128, 2], F32, kind="ExternalInput")
    lnw_d = nc.dram_tensor("lnw", [128, 2, 2], F32, kind="ExternalInput")
    lnb_d = nc.dram_tensor("lnb", [128, 2, 2], F32, kind="ExternalInput")
    out_d = [nc.dram_tensor(n_, [IPC, 2, 128, N], BF16, kind="ExternalOutput")
             for n_ in ("out0", "out1")]

    with tile.TileContext(nc) as tc:
        consts = tc.alloc_tile_pool(name="consts", bufs=1)
        inp = tc.alloc_tile_pool(name="inp", bufs=1)
        work = tc.alloc_tile_pool(name="work", bufs=2)
        psA = tc.alloc_tile_pool(name="psA", bufs=2, space="PSUM")
        psB = tc.alloc_tile_pool(name="psB", bufs=2, space="PSUM")

        # ---- constants; DMA'd on the scalar queue (idle at start)
        wq = [consts.tile([128, 2, 128], FP8, name=f"wq{s}", tag=f"wq{s}")
              for s in range(2)]
        wk = [consts.tile([128, 2, 128], FP8, name=f"wk{s}", tag=f"wk{s}")
              for s in range(2)]
        wv = [consts.tile([128, 2, 256], FP8, name=f"wv{s}", tag=f"wv{s}")
              for s in range(2)]
        wf8 = consts.tile([128, 4, 256], FP8, name="wf8", tag="wf8")
        fbias = consts.tile([128, 2], F32, name="fbias", tag="fbias")
        lnw = consts.tile([128, 2, 2], F32, name="lnw", tag="lnw")
        lnb = consts.tile([128, 2, 2], F32, name="lnb", tag="lnb")
        ones8 = consts.tile([128, 2, 128], FP8, name="ones8", tag="ones8")
        ones_col = consts.tile([128, 1], F32, name="ones_col", tag="ones_col")
        ones_row = consts.tile([1, 128], F32, name="ones_row", tag="ones_row")
        # stream 0 needs wq1/wk0/wv0 first — issue in that order
        nc.scalar.dma_start(out=wq[1][:], in_=wq_d[1][:])
        nc.scalar.dma_start(out=wk[0][:], in_=wk_d[0][:])
        nc.scalar.dma_start(out=wv[0][:], in_=wv_d[0][:])
        nc.scalar.dma_start(out=wq[0][:], in_=wq_d[0][:])
        nc.scalar.dma_start(out=wk[1][:], in_=wk_d[1][:])
        nc.scalar.dma_start(out=wv[1][:], in_=wv_d[1][:])
        nc.scalar.dma_start(out=wf8[:], in_=wf8_d[:])
        nc.scalar.dma_start(out=fbias[:], in_=fb_bias_d[:])
        nc.scalar.dma_start(out=lnw[:], in_=lnw_d[:])
        nc.scalar.dma_start(out=lnb[:], in_=lnb_d[:])
        nc.vector.memset(ones8[:], 1.0)
        nc.vector.memset(ones_col[:], 1.0)
        nc.vector.memset(ones_row[:], 1.0)

        # ---- prefetch ALL input tiles up front (48KB/partition total).
        # fp8 tiles first (the prologue conv needs fi8[0]/fs8[0] right
        # away); bf16 residual tiles trail (first used mid-body-0).
        fb_t = {}
        f8_t = {}
        for i in range(IPC):
            for s in (1, 0):
                t8 = inp.tile([128, 2, N], FP8, name=f"f8_{s}_{i}",
                              tag=f"f8_{s}_{i}")
                nc.sync.dma_start(out=t8[:], in_=f8_d[s][i])
                f8_t[(s, i)] = t8
        for i in range(IPC):
            for s in range(2):
                t = inp.tile([128, 2, N], BF16, name=f"fb{s}_{i}",
                             tag=f"fb{s}_{i}")
                nc.sync.dma_start(out=t[:], in_=fb_d[s][i])
                fb_t[(s, i)] = t

        def valid(x):
            return 0 <= x < NITER

        # -------- per-x tile state --------
        q_ps, k_ps, q_sb, k_sb = {}, {}, {}, {}
        vt_ps, vt_sb = {}, {}
        expS, pv_t, den_ps, rden, attn = {}, {}, {}, {}, {}
        psf, g_t, h_t = {}, {}, {}
        statsP, AB, MR = {}, {}, {}

        def ringB(nm):
            return psB.tile([128, N], F32, name=nm, tag="pvb", bufs=2)

        # -------- emit helpers --------
        def emit_q_mm(x):
            i, s = divmod(x, 2)
            for h in range(2):
                nc.tensor.matmul(q_ps[x][:, h * 512:(h + 1) * 512],
                                 lhsT=wq[1 - s][:],
                                 rhs=f8_t[(1 - s, i)][:, :,
                                                      h * 512:(h + 1) * 512],
                                 start=True, stop=True, perf_mode=DR)

        def emit_k_mm(x):
            i, s = divmod(x, 2)
            for h in range(2):
                nc.tensor.matmul(k_ps[x][:, h * 512:(h + 1) * 512],
                                 lhsT=wk[s][:],
                                 rhs=f8_t[(s, i)][:, :,
                                                  h * 512:(h + 1) * 512],
                                 start=True, stop=True, perf_mode=DR)

        def emit_q_evac(x):
            q_sb[x] = work.tile([128, N], BF16, name="q_sb", tag="q_sb",
                                bufs=3)
            nc.vector.tensor_copy(out=q_sb[x][:], in_=q_ps[x][:])

        def emit_k_evac(x):
            k_sb[x] = work.tile([128, N], BF16, name="k_sb", tag="k_sb",
                                bufs=3)
            nc.scalar.copy(out=k_sb[x][:], in_=k_ps[x][:])

        def emit_vt_mm(x, half):
            i, s = divmod(x, 2)
            for jj in range(4):
                j = half * 4 + jj
                nc.tensor.matmul(
                    vt_ps[x][half][:, jj * 256:(jj + 1) * 256],
                    lhsT=f8_t[(s, i)][:, :, j * 128:(j + 1) * 128],
                    rhs=wv[s][:], start=True, stop=True, perf_mode=DR)

        def emit_vt_evac(x, half):
            if half == 0:
                vt_sb[x] = work.tile([128, 8, 256], FP8, name="vt_sb",
                                     tag="vt", bufs=3)
            nc.vector.tensor_copy(
                out=vt_sb[x][:, half * 4:(half + 1) * 4, :]
                .rearrange("p a b -> p (a b)"),
                in_=vt_ps[x][half][:])

        def emit_S_exp(x, j):
            """S^T chunk j (PE, bf16) + exp (ACT) into expS[x][:, j, :]."""
            sl = psA.tile([128, N], F32, name="ps_s", tag="slab", bufs=2)
            for h in range(2):
                nc.tensor.matmul(sl[:, h * 512:(h + 1) * 512],
                                 lhsT=k_sb[x][:, j * 128:(j + 1) * 128],
                                 rhs=q_sb[x][:, h * 512:(h + 1) * 512],
                                 start=True, stop=True)
            nc.scalar.activation(out=expS[x][:, j, :], in_=sl[:],
                                 func=AF.Exp, scale=EXP_SCALE)

        def emit_S_mm(x, j):
            sl = psA.tile([128, N], F32, name="ps_s", tag="slab", bufs=2)
            for h in range(2):
                nc.tensor.matmul(sl[:, h * 512:(h + 1) * 512],
                                 lhsT=k_sb[x][:, j * 128:(j + 1) * 128],
                                 rhs=q_sb[x][:, h * 512:(h + 1) * 512],
                                 start=True, stop=True)
            return sl

        def emit_exp(x, j, sl):
            nc.scalar.activation(out=expS[x][:, j, :], in_=sl[:],
                                 func=AF.Exp, scale=EXP_SCALE)

        def emit_pv_jp(x, t, jp):
            for h in range(2):
                nc.tensor.matmul(
                    pv_t[x][t][:, h * 512:(h + 1) * 512],
                    lhsT=vt_sb[x][:, 2 * jp:2 * jp + 2,
                                  t * 128:(t + 1) * 128],
                    rhs=expS[x][:, 2 * jp:2 * jp + 2,
                                h * 512:(h + 1) * 512],
                    start=(jp == 0), stop=(jp == 3), perf_mode=DR)

        def emit_den_h(x, h):
            sl = slice(h * 512, (h + 1) * 512)
            for jp in range(4):
                nc.tensor.matmul(
                    den_ps[x][:, sl], lhsT=ones8[:],
                    rhs=expS[x][:, 2 * jp:2 * jp + 2, sl],
                    start=(jp == 0), stop=(jp == 3), perf_mode=DR)

        def emit_recip_h(x, h):
            if h == 0:
                rden[x] = work.tile([128, N], F32, name="rden", tag="rden",
                                    bufs=2)
            sl = slice(h * 512, (h + 1) * 512)
            nc.vector.reciprocal_approx_fast(out=rden[x][:, sl],
                                             in_=den_ps[x][:, sl])

        def emit_norm(x, t):
            if t == 0:
                attn[x] = work.tile([128, 2, N], FP8, name="attn_sb",
                                    tag="attn", bufs=2)
            nc.vector.tensor_tensor(out=attn[x][:, t, :],
                                    in0=pv_t[x][t][:], in1=rden[x][:],
                                    op=OP.mult)

        def emit_fuse_t(x, t):
            i, s = divmod(x, 2)
            for h in range(2):
                nc.tensor.matmul(
                    psf[x][t][:, h * 512:(h + 1) * 512],
                    lhsT=wf8[:, 0:2, t * 128:(t + 1) * 128],
                    rhs=f8_t[(s, i)][:, :, h * 512:(h + 1) * 512],
                    start=True, stop=False, perf_mode=DR)
            for h in range(2):
                nc.tensor.matmul(
                    psf[x][t][:, h * 512:(h + 1) * 512],
                    lhsT=wf8[:, 2:4, t * 128:(t + 1) * 128],
                    rhs=attn[x][:, :, h * 512:(h + 1) * 512],
                    start=False, stop=True, perf_mode=DR)

        def emit_relu(x, t):
            if t == 0:
                g_t[x] = work.tile([128, 2, N], BF16, name="g_t", tag="g_t",
                                   bufs=2)
            nc.scalar.activation(out=g_t[x][:, t, :], in_=psf[x][t][:],
                                 func=AF.Relu, bias=fbias[:, t:t + 1],
                                 scale=1.0 / (WSCALE * WSCALE))

        def emit_resid(x, t, eng):
            i, s = divmod(x, 2)
            if t == 0:
                h_t[x] = work.tile([128, 2, N], BF16, name="h_t", tag="h_t",
                                   bufs=5)
            eng.tensor_tensor(out=h_t[x][:, t, :], in0=g_t[x][:, t, :],
                              in1=fb_t[(s, i)][:, t, :], op=OP.add)

        st4_d = {}

        def emit_bn_half(x, t):
            """bn_stats records for t-half of h(x) (DVE)."""
            if t == 0:
                st4_d[x] = work.tile([128, 4, 6], F32, name="st4",
                                     tag="st4", bufs=2)
            st4 = st4_d[x]
            for hh in range(2):
                nc.vector.bn_stats(
                    out=st4[:, 2 * t + hh, :],
                    in_=h_t[x][:, t, hh * 512:(hh + 1) * 512])

        def emit_bn_red(x):
            """tiny reductions -> statsP[item] cols 3s..3s+2.

            Each bn_stats record covers 256 elements (even/odd halves of a
            512 block): per partition 8 records of (count, mean, M2)."""
            i, s = divmod(x, 2)
            st4 = st4_d[x]
            if s == 0:
                statsP[i] = work.tile([128, 6], F32, name="statsP",
                                      tag="statsP", bufs=2)
            m8 = st4[:, :, 1:6:3]   # [128, 4, 2] means
            M28 = st4[:, :, 2:6:3]  # [128, 4, 2] M2s
            sq8 = work.tile([128, 4, 2], F32, name="sq8", tag="sq8", bufs=2)
            nc.gpsimd.tensor_tensor(out=sq8[:], in0=m8, in1=m8, op=OP.mult)
            nc.vector.tensor_reduce(out=statsP[i][:, 3 * s:3 * s + 1],
                                    in_=m8, axis=AX.XY, op=OP.add)
            nc.vector.tensor_reduce(out=statsP[i][:, 3 * s + 1:3 * s + 2],
                                    in_=M28, axis=AX.XY, op=OP.add)
            nc.vector.tensor_reduce(out=statsP[i][:, 3 * s + 2:3 * s + 3],
                                    in_=sq8[:], axis=AX.XY, op=OP.add)

        def emit_chain(i, veng=None):
            """statsP[i] -> A/B per stream. PE ones-colsum + GpSimd tinies.

            Per stream s cols: [G=sum m, H1=sum M2, H2=sum m^2] over 1024
            records of 256 elems: mu = G/1024, E[x^2] = (H1 + 256*H2)/NTOT.
            """
            ps_st = psA.tile([1, 6], F32, name="ps_st", tag="slab", bufs=2)
            nc.tensor.matmul(ps_st[:], lhsT=ones_col[:], rhs=statsP[i][:],
                             start=True, stop=True)
            st = work.tile([1, 6], F32, name="st", tag="st", bufs=2)
            nc.vector.tensor_copy(out=st[:], in_=ps_st[:])
            # mom = [mu0, mu1, Ex2_0, Ex2_1]
            if veng is None:
                veng = nc.gpsimd
            mom = work.tile([1, 4], F32, name="mom", tag="mom", bufs=2)
            nc.vector.tensor_scalar(out=mom[:, 0:2], in0=st[:, 0:6:3],
                                    scalar1=1.0 / 1024.0, scalar2=None,
                                    op0=OP.mult)
            tmp6 = work.tile([1, 2], F32, name="tmp6", tag="tmp6", bufs=2)
            nc.vector.scalar_tensor_tensor(out=tmp6[:], in0=st[:, 2:6:3],
                                           scalar=256.0, in1=st[:, 1:6:3],
                                           op0=OP.mult, op1=OP.add)
            nc.vector.tensor_scalar(out=mom[:, 2:4], in0=tmp6[:],
                                    scalar1=1.0 / NTOT, scalar2=None,
                                    op0=OP.mult)
            # var = Ex2 - mu^2 + eps
            var = work.tile([1, 2], F32, name="var", tag="var", bufs=2)
            veng.tensor_tensor(out=var[:], in0=mom[:, 0:2],
                                    in1=mom[:, 0:2], op=OP.mult)
            nc.vector.scalar_tensor_tensor(out=var[:], in0=var[:],
                                           scalar=-1.0, in1=mom[:, 2:4],
                                           op0=OP.mult, op1=OP.add)
            veng.tensor_scalar(out=var[:], in0=var[:], scalar1=EPS,
                                    scalar2=None, op0=OP.add)
            # mr = [rstd0, -mu0, rstd1, -mu1]; Newton rsqrt (seed 0.92
            # converges for var in [0.3, 3.4]; LN var here is ~1.1)
            mr = work.tile([1, 4], F32, name="mr", tag="mr", bufs=2)
            y = mr[:, 0:4:2]
            veng.memset(y, 0.92)
            t1 = work.tile([1, 2], F32, name="t1", tag="t1", bufs=2)
            for _ in range(3):
                veng.tensor_tensor(out=t1[:], in0=y, in1=y, op=OP.mult)
                veng.tensor_tensor(out=t1[:], in0=var[:], in1=t1[:],
                                        op=OP.mult)
                veng.tensor_scalar(out=t1[:], in0=t1[:], scalar1=-0.5,
                                        scalar2=1.5, op0=OP.mult, op1=OP.add)
                veng.tensor_tensor(out=y, in0=y, in1=t1[:], op=OP.mult)
            veng.tensor_scalar(out=mr[:, 1:4:2], in0=mom[:, 0:2],
                                    scalar1=-1.0, scalar2=None, op0=OP.mult)
            MR[i] = mr

        def emit_chain_b(i):
            """mr -> mrb (PE K=1 broadcast; a GpSimd partition_broadcast
            would thrash the Pool ucode library ~6.7us/LOAD_LIB) -> A/B.
            Runs one body after emit_chain so the Newton tail never gates
            the ring-A rotation."""
            mr = MR[i]
            mrb_ps = psA.tile([128, 4], F32, name="mrb_ps", tag="slab",
                              bufs=2)
            nc.tensor.matmul(mrb_ps[:], lhsT=ones_row[:], rhs=mr[:],
                             start=True, stop=True)
            mrb = work.tile([128, 4], F32, name="mrb", tag="mrb", bufs=2)
            nc.vector.tensor_copy(out=mrb[:], in_=mrb_ps[:])
            # per-partition AP scalars are not supported on Pool — DVE
            Asb = work.tile([128, 2, 2], F32, name="Asb", tag="Asb", bufs=2)
            Bsb = work.tile([128, 2, 2], F32, name="Bsb", tag="Bsb", bufs=2)
            for s in range(2):
                nc.vector.tensor_scalar(
                    out=Asb[:, s, :], in0=lnw[:, s, :],
                    scalar1=mrb[:, 2 * s:2 * s + 1], scalar2=None,
                    op0=OP.mult)
                nc.vector.scalar_tensor_tensor(
                    out=Bsb[:, s, :], in0=Asb[:, s, :],
                    scalar=mrb[:, 2 * s + 1:2 * s + 2],
                    in1=lnb[:, s, :], op0=OP.mult, op1=OP.add)
            AB[i] = (Asb, Bsb)

        def emit_apply(x):
            i, s = divmod(x, 2)
            Asb, Bsb = AB[i]
            for t in range(2):
                o_t = work.tile([128, N], BF16, name="o_t", tag="o_t", bufs=4)
                nc.vector.tensor_scalar(
                    out=o_t[:], in0=h_t[x][:, t, :],
                    scalar1=Asb[:, s, t:t + 1], scalar2=Bsb[:, s, t:t + 1],
                    op0=OP.mult, op1=OP.add)
                nc.sync.dma_start(out=out_d[s][i, t], in_=o_t[:])

        # ================= prologue: conv(0) =================
        q_ps[0] = ringB("psq")
        k_ps[0] = ringB("psk")
        vt_ps[0] = [ringB("psvt0"), ringB("psvt1")]
        emit_q_mm(0)
        emit_q_evac(0)
        emit_k_mm(0)
        emit_k_evac(0)
        emit_vt_mm(0, 0)
        emit_vt_evac(0, 0)
        emit_vt_mm(0, 1)
        emit_vt_evac(0, 1)

        # ================= main pipeline =================
        for x in range(NITER + 4):
            xm1, xm2, xm4, xp1 = x - 1, x - 2, x - 4, x + 1
            sx = valid(x)       # S/exp/PV/den phase for x
            tl1 = valid(xm1)    # recip/norm/fuse/relu/resid for x-1
            tl2 = valid(xm2)    # bn stats for x-2
            cnv = valid(xp1)    # conv for x+1

            if sx:
                expS[x] = work.tile([128, 8, N], FP8, name="expS",
                                    tag="expS", bufs=3)
            # ---- ring B allocations (ORDER IS THE SCHEDULE) ----
            if tl1:
                psf[xm1] = [ringB("ps_f0"), ringB("ps_f1")]
            if cnv:
                q_ps[xp1] = ringB("psq")
                k_ps[xp1] = ringB("psk")
            if sx:
                den_ps[x] = ringB("ps_d")
                pv_t[x] = [ringB("ps_pv0"), ringB("ps_pv1")]
            if cnv:
                vt_ps[xp1] = [ringB("psvt0"), ringB("psvt1")]

            # ---- emission (engines interleaved) ----
            if tl1:
                emit_recip_h(xm1, 0)                 # DVE
                emit_recip_h(xm1, 1)
            if sx:
                emit_S_exp(x, 0)                     # PE + ACT
                emit_S_exp(x, 1)
            if tl1:
                emit_pv_jp(xm1, 0, 3)                # PE (exp7(x-1) done)
                emit_norm(xm1, 0)                    # DVE
            if sx:
                emit_S_exp(x, 2)
                emit_vt_mm(x, 0)                     # PE (ring-gated)
                emit_vt_evac(x, 0)                   # DVE
                emit_S_exp(x, 3)
            if tl1:
                for jp in range(4):                  # PE: PV-t1(x-1)
                    emit_pv_jp(xm1, 1, jp)
                emit_norm(xm1, 1)                    # DVE (after PV-t1)
            if sx:
                emit_S_exp(x, 4)
                emit_vt_mm(x, 1)
                emit_vt_evac(x, 1)                   # DVE
            if x >= 4 and x % 2 == 0 and valid(x - 4):
                emit_chain_b((x - 4) // 2)           # PE tiny + DVE
            if tl1:
                emit_fuse_t(xm1, 0)                  # PE
                emit_relu(xm1, 0)                    # ACT (after exp4)
            if sx:
                emit_S_exp(x, 5)
            if tl1:
                emit_fuse_t(xm1, 1)                  # PE
            if cnv:
                emit_q_mm(xp1)                       # PE
                emit_q_evac(xp1)                     # DVE
            if tl1:
                emit_relu(xm1, 1)                    # ACT (after exp5)
                r_eng = nc.vector if xm1 >= NITER - 2 else nc.gpsimd
                emit_resid(xm1, 0, r_eng)
                emit_resid(xm1, 1, r_eng)
            if sx:
                emit_S_exp(x, 6)
                sl7 = emit_S_mm(x, 7)                # PE (ahead of k-mm)
            if cnv:
                emit_k_mm(xp1)                       # PE
                emit_k_evac(xp1)                     # ACT (after exp6)
            if sx:
                emit_exp(x, 7, sl7)                  # ACT
                emit_den_h(x, 0)                     # PE (jp0-2 ready early)
                emit_den_h(x, 1)
                for jp in range(3):                  # PE: PV-t0(x) jp0..2
                    emit_pv_jp(x, 0, jp)
            if tl2:
                emit_bn_half(xm2, 0)                 # DVE
                emit_bn_half(xm2, 1)
                emit_bn_red(xm2)
            if x >= 3 and x % 2 == 1 and valid(xm2):
                ii = (x - 3) // 2
                emit_chain(ii, nc.vector if ii == IPC - 1 else None)
            if valid(xm4):
                emit_apply(xm4)                      # DVE + sync DMA

        psB.release()
        psA.release()
        work.release()
        inp.release()
        consts.release()

    nc.compile()
    return nc


_NC_CACHE = None


def _get_nc():
    global _NC_CACHE
    if _NC_CACHE is None:
        _NC_CACHE = _build()
    return _NC_CACHE


def kernel(fs, fi, qs_w, ks_w, vs_w, qi_w, ki_w, vi_w,
           fuse_w, fuse_b, ln_s_w, ln_s_b, ln_i_w, ln_i_b):
    global LAST_RESULT
    fs = np.asarray(fs, np.float32)
    fi = np.asarray(fi, np.float32)

    def prep_f(x):
        # (B, C, H, W) -> per-core [IPC, 128, 2, N] (partition-major so the
        # on-chip DMA is fully contiguous)
        x = x.reshape(NCORES, IPC, 2, 128, N)
        return np.ascontiguousarray(x.transpose(0, 1, 3, 2, 4))

    def prep_w_qk(w):  # (128, 256) -> lhsT layout [128p, 2kc, 128m] * 32
        wt = np.ascontiguousarray(np.asarray(w, np.float32).T) * WSCALE
        return np.ascontiguousarray(
            wt.reshape(2, 128, 128).transpose(1, 0, 2)).astype(
                ml_dtypes.float8_e4m3)

    def prep_w_v(w):  # (256, 256) -> rhs layout [128p, 2kc, 256c] * 32
        wt = np.ascontiguousarray(np.asarray(w, np.float32).T) * WSCALE
        return np.ascontiguousarray(
            wt.reshape(2, 128, 256).transpose(1, 0, 2)).astype(
                ml_dtypes.float8_e4m3)

    fs_sh = prep_f(fs)
    fi_sh = prep_f(fi)
    fs_bf = fs_sh.astype(ml_dtypes.bfloat16)
    fi_bf = fi_sh.astype(ml_dtypes.bfloat16)
    fs_q8 = fs_sh.astype(ml_dtypes.float8_e4m3)
    fi_q8 = fi_sh.astype(ml_dtypes.float8_e4m3)

    wq0 = prep_w_qk(qs_w)
    wq1 = prep_w_qk(qi_w)
    wk0 = prep_w_qk(ks_w)
    wk1 = prep_w_qk(ki_w)
    wv0 = prep_w_v(vs_w)
    wv1 = prep_w_v(vi_w)
    wfuse_t = np.ascontiguousarray(
        np.asarray(fuse_w, np.float32).T.reshape(4, 128, 256)
        .transpose(1, 0, 2))
    # f-half carries x1024 (f8 input at scale 1), attn-half x32 (attn8 is
    # 32x true attn) -> both matmul halves produce 1024x fuse pre-act.
    wfuse8 = np.concatenate(
        [wfuse_t[:, 0:2, :] * (WSCALE * WSCALE),
         wfuse_t[:, 2:4, :] * WSCALE], axis=1).astype(ml_dtypes.float8_e4m3)
    wfuse8 = np.ascontiguousarray(wfuse8)
    fuseb = np.ascontiguousarray(
        np.asarray(fuse_b, np.float32).reshape(2, 128).T)
    lnw = np.ascontiguousarray(
        np.stack([np.asarray(ln_s_w, np.float32).reshape(256),
                  np.asarray(ln_i_w, np.float32).reshape(256)])
        .reshape(2, 2, 128).transpose(2, 0, 1))
    lnb = np.ascontiguousarray(
        np.stack([np.asarray(ln_s_b, np.float32).reshape(256),
                  np.asarray(ln_i_b, np.float32).reshape(256)])
        .reshape(2, 2, 128).transpose(2, 0, 1))

    in_maps = []
    for c in range(NCORES):
        in_maps.append({
            "fsb": np.ascontiguousarray(fs_bf[c]),
            "fib": np.ascontiguousarray(fi_bf[c]),
            "fs8": np.ascontiguousarray(fs_q8[c]),
            "fi8": np.ascontiguousarray(fi_q8[c]),
            "wq0": wq0, "wq1": wq1, "wk0": wk0, "wk1": wk1,
            "wv0": wv0, "wv1": wv1, "wfuse8": wfuse8,
            "fuseb": fuseb, "lnw": lnw, "lnb": lnb,
        })

    nc = _get_nc()
    res = run_bass_kernel_spmd(nc, in_maps, core_ids=list(range(NCORES)),
                               **RUN_KWARGS)
    LAST_RESULT = res

    fs_out = np.empty((NCORES, IPC, 2, 128, N), np.float32)
    fi_out = np.empty((NCORES, IPC, 2, 128, N), np.float32)
    for c in range(NCORES):
        fs_out[c] = np.asarray(res.results[c]["out0"],
                               dtype=np.float32)
        fi_out[c] = np.asarray(res.results[c]["out1"],
                               dtype=np.float32)
    fs_out = fs_out.reshape(B, C, 32, 32)
    fi_out = fi_out.reshape(B, C, 32, 32)
    return fs_out, fi_out


# revision 31
# speedup vs baseline: 1.0063x; 1.0063x over previous
"""Trainium2 Bass kernel for nn_CrossAttention2d (B=32, C=256, INNER=128, H=W=32).

Sharding: pure data parallel — batch 32 split as 4 items per core across 8
NeuronCores; all weights replicated. No collectives.

Per item (N = H*W = 1024 tokens, C = 256 channels, D = 128 inner), stream s
(s=0 -> fs output, s=1 -> fi output):
  q = wq[1-s] @ f[1-s], k = wk[s] @ f[s]   (fp8 DoubleRow, x32 prescale)
  vT[m, c] = (wv[s] @ f[s]).T              (fp8 DoubleRow, f-slices stationary)
  S^T[m, n] = sum_d k[d, m] q[d, n]        (bf16 PE, m-tiles of 128)
  E = exp(S^T / (1024 sqrt(D)))            (ACT, [128,1024] psum slab -> fp8)
  O_un[c, n] = sum_m vT[m, c] E[m, n]      (fp8 DR, two sequential C-half
                                            passes through the shared ring)
  den[n] via ones.T @ E (fp8 DR, rows all equal den[n])
  attn8 = O_un * (1/den)                   (DVE, = 32x true attn, fp8)
  fuse: g = relu((W1*1024 @ f  +  W2*32 @ attn8) / 1024 + b)
  h = g + f[s] (bf16 residual, split DVE/GpSimd); LayerNorm over (C,N).
  LN stats: DVE bn_stats records -> per-partition [sum m, sum M2, sum m^2]
  -> PE ones-colsum -> GpSimd scalar chain (Newton rsqrt) -> GpSimd apply
  out = h * A + B (bf16; host widens to f32).

Software pipeline, one iteration x per stream, pitch ~11.5us. Per-engine
queues in steady state (x = current stream):
  ACT : exp(x) x8 with relu(x-1) x2 and k-evac(x+1) slotted in the gaps
  DVE : recip(x-1), norm(x-1) x2, vt-evac(x) x2, q-evac(x+1),
        resid(x-1) t0, bn_stats(x-2)
  GpS : resid(x-1) t1, LN chain (odd x), apply(x-3) + out DMA doorbells
  PE  : S(x) x8, PV-t1(x-1), vt(x), fuse(x-1), q/k(x+1), PV-t0(x), den(x)

PSUM (8 banks): ring A tag 'slab' 2x[128,1024] = S slabs (+ tiny chain
matmul); ring B tag 'pvb' 2x[128,1024] rotating per iteration:
  psf_t0(x-1), psf_t1(x-1), q(x+1), k(x+1), den(x), pv_t0(x), pv_t1(x),
  vt0(x+1), vt1(x+1)
The bufs=2 WAR chain of ring B self-schedules the pipeline: e.g. pv_t0(x)
waits k(x+1) evac, psf_t0(x) waits vt0(x+1) evac, etc.

Matmul convention: out[M, N] = lhsT.T @ rhs, lhsT = [K<=128, M<=128] (K on
partitions), rhs = [K, N<=512], out in PSUM f32 (bank-contained writes).
DoubleRow: lhsT [Ki, 2, M], rhs [Ki, 2, N] fp8 -> contracts 2*Ki.
"""

import numpy as np
import ml_dtypes

import concourse.bacc as bacc
import concourse.bass as bass
import concourse.tile as tile
from concourse import mybir
from concourse.bass_utils import run_bass_kernel_spmd

F32 = mybir.dt.float32
BF16 = mybir.dt.bfloat16
FP8 = mybir.dt.float8e4
DR = mybir.MatmulPerfMode.DoubleRow
AF = mybir.ActivationFunctionType
OP = mybir.AluOpType
AX = mybir.AxisListType

B, C, D, N = 32, 256, 128, 1024
NCORES = 8
IPC = B // NCORES  # items per core = 4
NITER = 2 * IPC    # stream iterations per core = 8
WSCALE = 32.0  # fp8 weight prescale (w*32 keeps N(0,0.02) in e4m3 range)
EXP_SCALE = (1.0 / float(np.sqrt(D))) / (WSCALE * WSCALE)
EPS = 1e-5
NTOT = float(C * N)  # layernorm element count per item/stream

# test.py can set {"trace": True}; harness path leaves this empty.
RUN_KWARGS = {}
LAST_RESULT = None


def _build():
    nc = bacc.Bacc("TRN2", target_bir_lowering=False, debug=False,
                   num_devices=NCORES)

    # ---- DRAM I/O (per-core shapes) ----
    fb_d = [nc.dram_tensor(n_, [IPC, 128, 2, N], BF16, kind="ExternalInput")
            for n_ in ("fsb", "fib")]
    f8_d = [nc.dram_tensor(n_, [IPC, 128, 2, N], FP8, kind="ExternalInput")
            for n_ in ("fs8", "fi8")]
    wq_d = [nc.dram_tensor(n_, [128, 2, 128], FP8, kind="ExternalInput")
            for n_ in ("wq0", "wq1")]
    wk_d = [nc.dram_tensor(n_, [128, 2, 128], FP8, kind="ExternalInput")
            for n_ in ("wk0", "wk1")]
    wv_d = [nc.dram_tensor(n_, [128, 2, 256], FP8, kind="ExternalInput")
            for n_ in ("wv0", "wv1")]
    wf8_d = nc.dram_tensor("wfuse8", [128, 4, 256], FP8, kind="ExternalInput")
    fb_bias_d = nc.dram_tensor("fuseb", [128, 2], F32, kind="ExternalInput")
    lnw_d = nc.dram_tensor("lnw", [128, 2, 2], F32, kind="ExternalInput")
    lnb_d = nc.dram_tensor("lnb", [128, 2, 2], F32, kind="ExternalInput")
    out_d = [nc.dram_tensor(n_, [IPC, 2, 128, N], BF16, kind="ExternalOutput")
             for n_ in ("out0", "out1")]

    with tile.TileContext(nc) as tc:
        consts = tc.alloc_tile_pool(name="consts", bufs=1)
        inp = tc.alloc_tile_pool(name="inp", bufs=1)
        work = tc.alloc_tile_pool(name="work", bufs=2)
        psA = tc.alloc_tile_pool(name="psA", bufs=2, space="PSUM")
        psB = tc.alloc_tile_pool(name="psB", bufs=2, space="PSUM")

        # ---- constants; DMA'd on the scalar queue (idle at start)
        wq = [consts.tile([128, 2, 128], FP8, name=f"wq{s}", tag=f"wq{s}")
              for s in range(2)]
        wk = [consts.tile([128, 2, 128], FP8, name=f"wk{s}", tag=f"wk{s}")
              for s in range(2)]
        wv = [consts.tile([128, 2, 256], FP8, name=f"wv{s}", tag=f"wv{s}")
              for s in range(2)]
        wf8 = consts.tile([128, 4, 256], FP8, name="wf8", tag="wf8")
        fbias = consts.tile([128, 2], F32, name="fbias", tag="fbias")
        lnw = consts.tile([128, 2, 2], F32, name="lnw", tag="lnw")
        lnb = consts.tile([128, 2, 2], F32, name="lnb", tag="lnb")
        ones8 = consts.tile([128, 2, 128], FP8, name="ones8", tag="ones8")
        ones_col = consts.tile([128, 1], F32, name="ones_col", tag="ones_col")
        ones_row = consts.tile([1, 128], F32, name="ones_row", tag="ones_row")
        # stream 0 needs wq1/wk0/wv0 first — issue in that order
        nc.scalar.dma_start(out=wq[1][:], in_=wq_d[1][:])
        nc.scalar.dma_start(out=wk[0][:], in_=wk_d[0][:])
        nc.scalar.dma_start(out=wv[0][:], in_=wv_d[0][:])
        nc.scalar.dma_start(out=wq[0][:], in_=wq_d[0][:])
        nc.scalar.dma_start(out=wk[1][:], in_=wk_d[1][:])
        nc.scalar.dma_start(out=wv[1][:], in_=wv_d[1][:])
        nc.scalar.dma_start(out=wf8[:], in_=wf8_d[:])
        nc.scalar.dma_start(out=fbias[:], in_=fb_bias_d[:])
        nc.scalar.dma_start(out=lnw[:], in_=lnw_d[:])
        nc.scalar.dma_start(out=lnb[:], in_=lnb_d[:])
        nc.vector.memset(ones8[:], 1.0)
        nc.vector.memset(ones_col[:], 1.0)
        nc.vector.memset(ones_row[:], 1.0)

        # ---- prefetch ALL input tiles up front (48KB/partition total).
        # fp8 tiles first (the prologue conv needs fi8[0]/fs8[0] right
        # away); bf16 residual tiles trail (first used mid-body-0).
        fb_t = {}
        f8_t = {}
        for i in range(IPC):
            for s in (1, 0):
                t8 = inp.tile([128, 2, N], FP8, name=f"f8_{s}_{i}",
                              tag=f"f8_{s}_{i}")
                nc.sync.dma_start(out=t8[:], in_=f8_d[s][i])
                f8_t[(s, i)] = t8
        for i in range(IPC):
            for s in range(2):
                t = inp.tile([128, 2, N], BF16, name=f"fb{s}_{i}",
                             tag=f"fb{s}_{i}")
                nc.sync.dma_start(out=t[:], in_=fb_d[s][i])
                fb_t[(s, i)] = t

        def valid(x):
            return 0 <= x < NITER

        # -------- per-x tile state --------
        q_ps, k_ps, q_sb, k_sb = {}, {}, {}, {}
        vt_ps, vt_sb = {}, {}
        expS, pv_t, den_ps, rden, attn = {}, {}, {}, {}, {}
        psf, g_t, h_t = {}, {}, {}
        statsP, AB, MR = {}, {}, {}

        def ringB(nm):
            return psB.tile([128, N], F32, name=nm, tag="pvb", bufs=2)

        # -------- emit helpers --------
        def emit_q_mm(x):
            i, s = divmod(x, 2)
            for h in range(2):
                nc.tensor.matmul(q_ps[x][:, h * 512:(h + 1) * 512],
                                 lhsT=wq[1 - s][:],
                                 rhs=f8_t[(1 - s, i)][:, :,
                                                      h * 512:(h + 1) * 512],
                                 start=True, stop=True, perf_mode=DR)

        def emit_k_mm(x):
            i, s = divmod(x, 2)
            for h in range(2):
                nc.tensor.matmul(k_ps[x][:, h * 512:(h + 1) * 512],
                                 lhsT=wk[s][:],
                                 rhs=f8_t[(s, i)][:, :,
                                                  h * 512:(h + 1) * 512],
                                 start=True, stop=True, perf_mode=DR)

        def emit_q_evac(x):
            q_sb[x] = work.tile([128, N], BF16, name="q_sb", tag="q_sb",
                                bufs=3)
            nc.vector.tensor_copy(out=q_sb[x][:], in_=q_ps[x][:])

        def emit_k_evac(x):
            k_sb[x] = work.tile([128, N], BF16, name="k_sb", tag="k_sb",
                                bufs=3)
            nc.scalar.copy(out=k_sb[x][:], in_=k_ps[x][:])

        def emit_vt_mm(x, half):
            i, s = divmod(x, 2)
            for jj in range(4):
                j = half * 4 + jj
                nc.tensor.matmul(
                    vt_ps[x][half][:, jj * 256:(jj + 1) * 256],
                    lhsT=f8_t[(s, i)][:, :, j * 128:(j + 1) * 128],
                    rhs=wv[s][:], start=True, stop=True, perf_mode=DR)

        def emit_vt_evac(x, half):
            if half == 0:
                vt_sb[x] = work.tile([128, 8, 256], FP8, name="vt_sb",
                                     tag="vt", bufs=3)
            nc.vector.tensor_copy(
                out=vt_sb[x][:, half * 4:(half + 1) * 4, :]
                .rearrange("p a b -> p (a b)"),
                in_=vt_ps[x][half][:])

        def emit_S_exp(x, j):
            """S^T chunk j (PE, bf16) + exp (ACT) into expS[x][:, j, :]."""
            sl = psA.tile([128, N], F32, name="ps_s", tag="slab", bufs=2)
            for h in range(2):
                nc.tensor.matmul(sl[:, h * 512:(h + 1) * 512],
                                 lhsT=k_sb[x][:, j * 128:(j + 1) * 128],
                                 rhs=q_sb[x][:, h * 512:(h + 1) * 512],
                                 start=True, stop=True)
            nc.scalar.activation(out=expS[x][:, j, :], in_=sl[:],
                                 func=AF.Exp, scale=EXP_SCALE)

        def emit_S_mm(x, j):
            sl = psA.tile([128, N], F32, name="ps_s", tag="slab", bufs=2)
            for h in range(2):
                nc.tensor.matmul(sl[:, h * 512:(h + 1) * 512],
                                 lhsT=k_sb[x][:, j * 128:(j + 1) * 128],
                                 rhs=q_sb[x][:, h * 512:(h + 1) * 512],
                                 start=True, stop=True)
            return sl

        def emit_exp(x, j, sl):
            nc.scalar.activation(out=expS[x][:, j, :], in_=sl[:],
                                 func=AF.Exp, scale=EXP_SCALE)

        def emit_pv_jp(x, t, jp):
            for h in range(2):
                nc.tensor.matmul(
                    pv_t[x][t][:, h * 512:(h + 1) * 512],
                    lhsT=vt_sb[x][:, 2 * jp:2 * jp + 2,
                                  t * 128:(t + 1) * 128],
                    rhs=expS[x][:, 2 * jp:2 * jp + 2,
                                h * 512:(h + 1) * 512],
                    start=(jp == 0), stop=(jp == 3), perf_mode=DR)

        def emit_den_h(x, h):
            sl = slice(h * 512, (h + 1) * 512)
            for jp in range(4):
                nc.tensor.matmul(
                    den_ps[x][:, sl], lhsT=ones8[:],
                    rhs=expS[x][:, 2 * jp:2 * jp + 2, sl],
                    start=(jp == 0), stop=(jp == 3), perf_mode=DR)

        def emit_recip_h(x, h):
            if h == 0:
                rden[x] = work.tile([128, N], F32, name="rden", tag="rden",
                                    bufs=2)
            sl = slice(h * 512, (h + 1) * 512)
            nc.vector.reciprocal_approx_fast(out=rden[x][:, sl],
                                             in_=den_ps[x][:, sl])

        def emit_norm(x, t):
            if t == 0:
                attn[x] = work.tile([128, 2, N], FP8, name="attn_sb",
                                    tag="attn", bufs=2)
            nc.vector.tensor_tensor(out=attn[x][:, t, :],
                                    in0=pv_t[x][t][:], in1=rden[x][:],
                                    op=OP.mult)

        def emit_fuse_t(x, t):
            i, s = divmod(x, 2)
            for h in range(2):
                nc.tensor.matmul(
                    psf[x][t][:, h * 512:(h + 1) * 512],
                    lhsT=wf8[:, 0:2, t * 128:(t + 1) * 128],
                    rhs=f8_t[(s, i)][:, :, h * 512:(h + 1) * 512],
                    start=True, stop=False, perf_mode=DR)
            for h in range(2):
                nc.tensor.matmul(
                    psf[x][t][:, h * 512:(h + 1) * 512],
                    lhsT=wf8[:, 2:4, t * 128:(t + 1) * 128],
                    rhs=attn[x][:, :, h * 512:(h + 1) * 512],
                    start=False, stop=True, perf_mode=DR)

        def emit_relu(x, t):
            if t == 0:
                g_t[x] = work.tile([128, 2, N], BF16, name="g_t", tag="g_t",
                                   bufs=2)
            nc.scalar.activation(out=g_t[x][:, t, :], in_=psf[x][t][:],
                                 func=AF.Relu, bias=fbias[:, t:t + 1],
                                 scale=1.0 / (WSCALE * WSCALE))

        def emit_resid(x, t, eng):
            i, s = divmod(x, 2)
            if t == 0:
                h_t[x] = work.tile([128, 2, N], BF16, name="h_t", tag="h_t",
                                   bufs=5)
            eng.tensor_tensor(out=h_t[x][:, t, :], in0=g_t[x][:, t, :],
                              in1=fb_t[(s, i)][:, t, :], op=OP.add)

        st4_d = {}

        def emit_bn_half(x, t):
            """bn_stats records for t-half of h(x) (DVE)."""
            if t == 0:
                st4_d[x] = work.tile([128, 4, 6], F32, name="st4",
                                     tag="st4", bufs=2)
            st4 = st4_d[x]
            for hh in range(2):
                nc.vector.bn_stats(
                    out=st4[:, 2 * t + hh, :],
                    in_=h_t[x][:, t, hh * 512:(hh + 1) * 512])

        def emit_bn_red(x):
            """tiny reductions -> statsP[item] cols 3s..3s+2.

            Each bn_stats record covers 256 elements (even/odd halves of a
            512 block): per partition 8 records of (count, mean, M2)."""
            i, s = divmod(x, 2)
            st4 = st4_d[x]
            if s == 0:
                statsP[i] = work.tile([128, 6], F32, name="statsP",
                                      tag="statsP", bufs=2)
            m8 = st4[:, :, 1:6:3]   # [128, 4, 2] means
            M28 = st4[:, :, 2:6:3]  # [128, 4, 2] M2s
            sq8 = work.tile([128, 4, 2], F32, name="sq8", tag="sq8", bufs=2)
            nc.gpsimd.tensor_tensor(out=sq8[:], in0=m8, in1=m8, op=OP.mult)
            nc.vector.tensor_reduce(out=statsP[i][:, 3 * s:3 * s + 1],
                                    in_=m8, axis=AX.XY, op=OP.add)
            nc.vector.tensor_reduce(out=statsP[i][:, 3 * s + 1:3 * s + 2],
                                    in_=M28, axis=AX.XY, op=OP.add)
            nc.vector.tensor_reduce(out=statsP[i][:, 3 * s + 2:3 * s + 3],
                                    in_=sq8[:], axis=AX.XY, op=OP.add)

        def emit_chain(i, veng=None):
            """statsP[i] -> A/B per stream. PE ones-colsum + GpSimd tinies.

            Per stream s cols: [G=sum m, H1=sum M2, H2=sum m^2] over 1024
            records of 256 elems: mu = G/1024, E[x^2] = (H1 + 256*H2)/NTOT.
            """
            ps_st = psA.tile([1, 6], F32, name="ps_st", tag="slab", bufs=2)
            nc.tensor.matmul(ps_st[:], lhsT=ones_col[:], rhs=statsP[i][:],
                             start=True, stop=True)
            st = work.tile([1, 6], F32, name="st", tag="st", bufs=2)
            nc.vector.tensor_copy(out=st[:], in_=ps_st[:])
            # mom = [mu0, mu1, Ex2_0, Ex2_1]
            if veng is None:
                veng = nc.gpsimd
            mom = work.tile([1, 4], F32, name="mom", tag="mom", bufs=2)
            nc.vector.tensor_scalar(out=mom[:, 0:2], in0=st[:, 0:6:3],
                                    scalar1=1.0 / 1024.0, scalar2=None,
                                    op0=OP.mult)
            tmp6 = work.tile([1, 2], F32, name="tmp6", tag="tmp6", bufs=2)
            nc.vector.scalar_tensor_tensor(out=tmp6[:], in0=st[:, 2:6:3],
                                           scalar=256.0, in1=st[:, 1:6:3],
                                           op0=OP.mult, op1=OP.add)
            nc.vector.tensor_scalar(out=mom[:, 2:4], in0=tmp6[:],
                                    scalar1=1.0 / NTOT, scalar2=None,
                                    op0=OP.mult)
            # var = Ex2 - mu^2 + eps
            var = work.tile([1, 2], F32, name="var", tag="var", bufs=2)
            veng.tensor_tensor(out=var[:], in0=mom[:, 0:2],
                                    in1=mom[:, 0:2], op=OP.mult)
            nc.vector.scalar_tensor_tensor(out=var[:], in0=var[:],
                                           scalar=-1.0, in1=mom[:, 2:4],
                                           op0=OP.mult, op1=OP.add)
            veng.tensor_scalar(out=var[:], in0=var[:], scalar1=EPS,
                                    scalar2=None, op0=OP.add)
            # mr = [rstd0, -mu0, rstd1, -mu1]; Newton rsqrt (seed 0.92
            # converges for var in [0.3, 3.4]; LN var here is ~1.1)
            mr = work.tile([1, 4], F32, name="mr", tag="mr", bufs=2)
            y = mr[:, 0:4:2]
            veng.memset(y, 0.92)
            t1 = work.tile([1, 2], F32, name="t1", tag="t1", bufs=2)
            for _ in range(3):
                veng.tensor_tensor(out=t1[:], in0=y, in1=y, op=OP.mult)
                veng.tensor_tensor(out=t1[:], in0=var[:], in1=t1[:],
                                        op=OP.mult)
                veng.tensor_scalar(out=t1[:], in0=t1[:], scalar1=-0.5,
                                        scalar2=1.5, op0=OP.mult, op1=OP.add)
                veng.tensor_tensor(out=y, in0=y, in1=t1[:], op=OP.mult)
            veng.tensor_scalar(out=mr[:, 1:4:2], in0=mom[:, 0:2],
                                    scalar1=-1.0, scalar2=None, op0=OP.mult)
            MR[i] = mr

        def emit_chain_b(i):
            """mr -> mrb (PE K=1 broadcast; a GpSimd partition_broadcast
            would thrash the Pool ucode library ~6.7us/LOAD_LIB) -> A/B.
            Runs one body after emit_chain so the Newton tail never gates
            the ring-A rotation."""
            mr = MR[i]
            mrb_ps = psA.tile([128, 4], F32, name="mrb_ps", tag="slab",
                              bufs=2)
            nc.tensor.matmul(mrb_ps[:], lhsT=ones_row[:], rhs=mr[:],
                             start=True, stop=True)
            mrb = work.tile([128, 4], F32, name="mrb", tag="mrb", bufs=2)
            nc.vector.tensor_copy(out=mrb[:], in_=mrb_ps[:])
            # per-partition AP scalars are not supported on Pool — DVE
            Asb = work.tile([128, 2, 2], F32, name="Asb", tag="Asb", bufs=2)
            Bsb = work.tile([128, 2, 2], F32, name="Bsb", tag="Bsb", bufs=2)
            for s in range(2):
                nc.vector.tensor_scalar(
                    out=Asb[:, s, :], in0=lnw[:, s, :],
                    scalar1=mrb[:, 2 * s:2 * s + 1], scalar2=None,
                    op0=OP.mult)
                nc.vector.scalar_tensor_tensor(
                    out=Bsb[:, s, :], in0=Asb[:, s, :],
                    scalar=mrb[:, 2 * s + 1:2 * s + 2],
                    in1=lnb[:, s, :], op0=OP.mult, op1=OP.add)
            AB[i] = (Asb, Bsb)

        def emit_apply(x):
            i, s = divmod(x, 2)
            Asb, Bsb = AB[i]
            for t in range(2):
                o_t = work.tile([128, N], BF16, name="o_t", tag="o_t", bufs=4)
                nc.vector.tensor_scalar(
                    out=o_t[:], in0=h_t[x][:, t, :],
                    scalar1=Asb[:, s, t:t + 1], scalar2=Bsb[:, s, t:t + 1],
                    op0=OP.mult, op1=OP.add)
                nc.sync.dma_start(out=out_d[s][i, t], in_=o_t[:])

        # ================= prologue: conv(0) =================
        q_ps[0] = ringB("psq")
        k_ps[0] = ringB("psk")
        vt_ps[0] = [ringB("psvt0"), ringB("psvt1")]
        emit_q_mm(0)
        emit_q_evac(0)
        emit_k_mm(0)
        emit_k_evac(0)
        emit_vt_mm(0, 0)
        emit_vt_evac(0, 0)
        emit_vt_mm(0, 1)
        emit_vt_evac(0, 1)

        # ================= main pipeline =================
        for x in range(NITER + 4):
            xm1, xm2, xm4, xp1 = x - 1, x - 2, x - 4, x + 1
            sx = valid(x)       # S/exp/PV/den phase for x
            tl1 = valid(xm1)    # recip/norm/fuse/relu/resid for x-1
            tl2 = valid(xm2)    # bn stats for x-2
            cnv = valid(xp1)    # conv for x+1

            if sx:
                expS[x] = work.tile([128, 8, N], FP8, name="expS",
                                    tag="expS", bufs=3)
            # ---- ring B allocations (ORDER IS THE SCHEDULE) ----
            if tl1:
                psf[xm1] = [ringB("ps_f0"), ringB("ps_f1")]
            if cnv:
                q_ps[xp1] = ringB("psq")
                k_ps[xp1] = ringB("psk")
            if sx:
                den_ps[x] = ringB("ps_d")
                pv_t[x] = [ringB("ps_pv0"), ringB("ps_pv1")]
            if cnv:
                vt_ps[xp1] = [ringB("psvt0"), ringB("psvt1")]

            # ---- emission (engines interleaved) ----
            if tl1:
                emit_recip_h(xm1, 0)                 # DVE
                emit_recip_h(xm1, 1)
            if sx:
                emit_S_exp(x, 0)                     # PE + ACT
                emit_S_exp(x, 1)
            if tl1:
                emit_norm(xm1, 0)                    # DVE
            if sx:
                emit_S_exp(x, 2)
                emit_vt_mm(x, 0)                     # PE (ring-gated)
                emit_vt_evac(x, 0)                   # DVE
                emit_S_exp(x, 3)
            if tl1:
                for jp in range(4):                  # PE: PV-t1(x-1)
                    emit_pv_jp(xm1, 1, jp)
                emit_norm(xm1, 1)                    # DVE (after PV-t1)
            if sx:
                emit_S_exp(x, 4)
                emit_vt_mm(x, 1)
                emit_vt_evac(x, 1)                   # DVE
            if x >= 4 and x % 2 == 0 and valid(x - 4):
                emit_chain_b((x - 4) // 2)           # PE tiny + DVE
            if tl1:
                emit_fuse_t(xm1, 0)                  # PE
                emit_relu(xm1, 0)                    # ACT (after exp4)
            if sx:
                emit_S_exp(x, 5)
            if tl1:
                emit_fuse_t(xm1, 1)                  # PE
            if cnv:
                emit_q_mm(xp1)                       # PE
                emit_q_evac(xp1)                     # DVE
            if tl1:
                emit_relu(xm1, 1)                    # ACT (after exp5)
                r_eng = nc.vector if xm1 >= NITER - 2 else nc.gpsimd
                emit_resid(xm1, 0, r_eng)
                emit_resid(xm1, 1, r_eng)
            if sx:
                emit_S_exp(x, 6)
                sl7 = emit_S_mm(x, 7)                # PE (ahead of k-mm)
            if cnv:
                emit_k_mm(xp1)                       # PE
                emit_k_evac(xp1)                     # ACT (after exp6)
            if sx:
                emit_exp(x, 7, sl7)                  # ACT
                for jp in range(3):                  # PE: PV-t0(x) jp0..2
                    emit_pv_jp(x, 0, jp)
                emit_den_h(x, 0)                     # PE (after exp7)
                emit_den_h(x, 1)
                emit_pv_jp(x, 0, 3)                  # PE tail
            if tl2:
                emit_bn_half(xm2, 0)                 # DVE
                emit_bn_half(xm2, 1)
                emit_bn_red(xm2)
            if x >= 3 and x % 2 == 1 and valid(xm2):
                ii = (x - 3) // 2
                emit_chain(ii, nc.vector if ii == IPC - 1 else None)
            if valid(xm4):
                emit_apply(xm4)                      # DVE + sync DMA

        psB.release()
        psA.release()
        work.release()
        inp.release()
        consts.release()

    nc.compile()
    return nc


_NC_CACHE = None


def _get_nc():
    global _NC_CACHE
    if _NC_CACHE is None:
        _NC_CACHE = _build()
    return _NC_CACHE


def kernel(fs, fi, qs_w, ks_w, vs_w, qi_w, ki_w, vi_w,
           fuse_w, fuse_b, ln_s_w, ln_s_b, ln_i_w, ln_i_b):
    global LAST_RESULT
    fs = np.asarray(fs, np.float32)
    fi = np.asarray(fi, np.float32)

    def prep_f(x):
        # (B, C, H, W) -> per-core [IPC, 128, 2, N] (partition-major so the
        # on-chip DMA is fully contiguous)
        x = x.reshape(NCORES, IPC, 2, 128, N)
        return np.ascontiguousarray(x.transpose(0, 1, 3, 2, 4))

    def prep_w_qk(w):  # (128, 256) -> lhsT layout [128p, 2kc, 128m] * 32
        wt = np.ascontiguousarray(np.asarray(w, np.float32).T) * WSCALE
        return np.ascontiguousarray(
            wt.reshape(2, 128, 128).transpose(1, 0, 2)).astype(
                ml_dtypes.float8_e4m3)

    def prep_w_v(w):  # (256, 256) -> rhs layout [128p, 2kc, 256c] * 32
        wt = np.ascontiguousarray(np.asarray(w, np.float32).T) * WSCALE
        return np.ascontiguousarray(
            wt.reshape(2, 128, 256).transpose(1, 0, 2)).astype(
                ml_dtypes.float8_e4m3)

    fs_sh = prep_f(fs)
    fi_sh = prep_f(fi)
    fs_bf = fs_sh.astype(ml_dtypes.bfloat16)
    fi_bf = fi_sh.astype(ml_dtypes.bfloat16)
    fs_q8 = fs_sh.astype(ml_dtypes.float8_e4m3)
    fi_q8 = fi_sh.astype(ml_dtypes.float8_e4m3)

    wq0 = prep_w_qk(qs_w)
    wq1 = prep_w_qk(qi_w)
    wk0 = prep_w_qk(ks_w)
    wk1 = prep_w_qk(ki_w)
    wv0 = prep_w_v(vs_w)
    wv1 = prep_w_v(vi_w)
    wfuse_t = np.ascontiguousarray(
        np.asarray(fuse_w, np.float32).T.reshape(4, 128, 256)
        .transpose(1, 0, 2))
    # f-half carries x1024 (f8 input at scale 1), attn-half x32 (attn8 is
    # 32x true attn) -> both matmul halves produce 1024x fuse pre-act.
    wfuse8 = np.concatenate(
        [wfuse_t[:, 0:2, :] * (WSCALE * WSCALE),
         wfuse_t[:, 2:4, :] * WSCALE], axis=1).astype(ml_dtypes.float8_e4m3)
    wfuse8 = np.ascontiguousarray(wfuse8)
    fuseb = np.ascontiguousarray(
        np.asarray(fuse_b, np.float32).reshape(2, 128).T)
    lnw = np.ascontiguousarray(
        np.stack([np.asarray(ln_s_w, np.float32).reshape(256),
                  np.asarray(ln_i_w, np.float32).reshape(256)])
        .reshape(2, 2, 128).transpose(2, 0, 1))
    lnb = np.ascontiguousarray(
        np.stack([np.asarray(ln_s_b, np.float32).reshape(256),
                  np.asarray(ln_i_b, np.float32).reshape(256)])
        .reshape(2, 2, 128).transpose(2, 0, 1))

    in_maps = []
    for c in range(NCORES):
        in_maps.append({
            "fsb": np.ascontiguousarray(fs_bf[c]),
            "fib": np.ascontiguousarray(fi_bf[c]),
            "fs8": np.ascontiguousarray(fs_q8[c]),
            "fi8": np.ascontiguousarray(fi_q8[c]),
            "wq0": wq0, "wq1": wq1, "wk0": wk0, "wk1": wk1,
            "wv0": wv0, "wv1": wv1, "wfuse8": wfuse8,
            "fuseb": fuseb, "lnw": lnw, "lnb": lnb,
        })

    nc = _get_nc()
    res = run_bass_kernel_spmd(nc, in_maps, core_ids=list(range(NCORES)),
                               **RUN_KWARGS)
    LAST_RESULT = res

    fs_out = np.empty((NCORES, IPC, 2, 128, N), np.float32)
    fi_out = np.empty((NCORES, IPC, 2, 128, N), np.float32)
    for c in range(NCORES):
        fs_out[c] = np.asarray(res.results[c]["out0"],
                               dtype=np.float32)
        fi_out[c] = np.asarray(res.results[c]["out1"],
                               dtype=np.float32)
    fs_out = fs_out.reshape(B, C, 32, 32)
    fi_out = fi_out.reshape(B, C, 32, 32)
    return fs_out, fi_out
